# revision 1
# baseline (speedup 1.0000x reference)
"""LISTA scan kernel for 8 TRN2 NeuronCores.

Strategy (tensor-parallel over the m=2048 code dimension, 256 rows/core):
  - B_k h is computed factored as A_k (A_k^T h)  (rank-512, half the FLOPs).
  - Per k-step:  s_c = P_k^T h_c  (PE, bf16)  ->  all-gather of the [512]
    partial via 8 single-dest remote_dma_broadcasts (XOR-relative routing,
    all compile-time-constant addresses)  ->  DVE 3-op tree-sum -> u (bf16)
    ->  psum_v = I @ (h + c_tk) - (A_k/a) u  (PE)  ->  h' = relu(psum_v) (ACT).
  - Phase A precomputes c[t,k,:] = (x @ (A_k/a_k)^T) - rho/a_k on device
    as a plain GEMM, stored to DRAM in the scan's per-partition layout.
  - Raw bacc, manual semaphores, hardware Fori loop per 512-t chunk with
    2 timesteps unrolled per iteration.
"""
import sys
import numpy as np

sys.path.insert(0, '/opt/trn_rl_repo')

from concourse import bass, bacc, mybir  # noqa: E402

T, N, M, K = 8192, 512, 2048, 3
RHO = 1e-4
NCORES = 8
MSL = M // NCORES            # 256 rows per core
JT = N // 128                # 4 u tiles
IT = MSL // 128              # 2 h tiles per core

F32 = mybir.dt.float32
BF16 = mybir.dt.bfloat16


def build_program(t_total, chunk_t, bias_vals):
    nchunk = t_total // chunk_t
    n_body = chunk_t // 2

    PE = mybir.EngineType.PE
    DVE = mybir.EngineType.DVE
    ACT = mybir.EngineType.Activation
    POOL = mybir.EngineType.Pool

    nc = bacc.Bacc(None, target_bir_lowering=False)

    # ---------------- DRAM ----------------
    xT = nc.declare_dram_parameter("xT", [JT, 128, t_total], F32, isOutput=False)
    wA = nc.declare_dram_parameter("wA", [128, K * JT * IT * 128], F32, isOutput=False)
    wS = nc.declare_dram_parameter("wS", [128, K * IT * JT * 128], BF16, isOutput=False)
    wV = nc.declare_dram_parameter("wV", [128, K * JT * IT * 128], BF16, isOutput=False)
    ident = nc.declare_dram_parameter("ident", [128, 128], BF16, isOutput=False)
    h0c = nc.declare_dram_parameter("h0c", [128, IT], BF16, isOutput=False)
    hs = nc.declare_dram_parameter("hs", [nchunk, 128, chunk_t * IT], BF16,
                                   isOutput=True)
    cdram = nc.dram_tensor("cdram", [nchunk, 128, chunk_t * 2 * K], BF16)

    # ---------------- SBUF ----------------
    ws_s = nc.alloc_sbuf_tensor("ws_s", [128, K * IT * JT * 128], BF16)
    ws_v = nc.alloc_sbuf_tensor("ws_v", [128, K * JT * IT * 128], BF16)
    wa_s = nc.alloc_sbuf_tensor("wa_s", [128, K * JT * IT * 128], F32)
    idn = nc.alloc_sbuf_tensor("idn", [128, 128], BF16)
    h = nc.alloc_sbuf_tensor("h", [128, IT], BF16)
    xch = [nc.alloc_sbuf_tensor(f"xch{p}", [128, JT * chunk_t], F32)
           for p in range(2)]
    pastage = [nc.alloc_sbuf_tensor(f"pastage{p}", [128, chunk_t, 2 * K], BF16)
               for p in range(2)]
    cbuf = [nc.alloc_sbuf_tensor(f"cbuf{p}", [128, chunk_t * 2 * K], BF16)
            for p in range(2)]
    hcb = [nc.alloc_sbuf_tensor(f"hc{p}", [128, IT], BF16) for p in range(2)]
    send = [nc.alloc_sbuf_tensor(f"send{p}", [128, JT], F32) for p in range(2)]
    recv = [nc.alloc_sbuf_tensor(f"recv{p}", [128, NCORES * JT], F32)
            for p in range(2)]
    ubuf = [nc.alloc_sbuf_tensor(f"ubuf{p}", [128, JT], BF16) for p in range(2)]
    ostage = [nc.alloc_sbuf_tensor(f"ostage{p}", [128, chunk_t * IT], BF16)
              for p in range(2)]

    # ---------------- semaphores ----------------
    names = ["ld_sem", "pe_tc", "dv_tc", "psem", "s_h", "s_hc",
             "s_u", "s_sd", "s_vd", "s_sc", "s_oc", "s_sf"]
    sems = {n: nc.alloc_semaphore(n) for n in names}
    (ld_sem, pe_tc, dv_tc, psem, s_h, s_hc, s_u, s_sd, s_vd,
     s_sc, s_oc, s_sf) = (sems[n] for n in names)
    lsem = [nc.alloc_semaphore(f"lsem{p}") for p in range(2)]
    # parity-split sems: multiple unordered DMA producers would otherwise
    # make thresholds ambiguous (a fast producer of round K+2 could satisfy
    # a round-K wait).
    xdma = [nc.alloc_semaphore(f"xdma{p}") for p in range(2)]
    st_out = [nc.alloc_semaphore(f"st_out{p}") for p in range(2)]
    csem = [nc.alloc_semaphore(f"csem{p}") for p in range(2)]
    osem = [nc.alloc_semaphore(f"osem{p}") for p in range(2)]
    rsem = [nc.alloc_semaphore(f"rsem{p}") for p in range(2)]

    te, ve, se, po, sp = nc.tensor, nc.vector, nc.scalar, nc.gpsimd, nc.sync

    # ---------------- entry loads ----------------
    sp.dma_start(out=ws_s[:, :], in_=wS[:, :]).then_inc(ld_sem, 16)
    sp.dma_start(out=ws_v[:, :], in_=wV[:, :]).then_inc(ld_sem, 16)
    sp.dma_start(out=wa_s[:, :], in_=wA[:, :]).then_inc(ld_sem, 16)
    sp.dma_start(out=idn[:, :], in_=ident[:, :]).then_inc(ld_sem, 16)
    sp.dma_start(out=h[:, :], in_=h0c[:, :]).then_inc(ld_sem, 16)

    # ================= PHASE A =================
    import contextlib
    with contextlib.ExitStack() as stack:
        psPA = [stack.enter_context(
            nc.psum_tensor(f"psPA{q}", [128, chunk_t], F32)) for q in range(6)]
        te.wait_ge(ld_sem, 80)
        for tc in range(nchunk):
            par = tc % 2
            if tc >= 2:
                sp.wait_ge(pe_tc, tc - 1)
            for nt in range(JT):
                sp.dma_start(out=xch[par][:, nt * chunk_t:(nt + 1) * chunk_t],
                             in_=xT[nt, :, tc * chunk_t:(tc + 1) * chunk_t]
                             ).then_inc(xdma[par], 16)
            te.wait_ge(xdma[par], 64 * (tc // 2 + 1))
            if tc >= 1:
                te.wait_ge(dv_tc, tc)
            last = None
            for k in range(K):
                for ic in range(IT):
                    for nt in range(JT):
                        wtile = ((k * JT + nt) * IT + ic) * 128
                        last = te.matmul(
                            psPA[k * IT + ic][:, :],
                            lhsT=wa_s[:, wtile:wtile + 128],
                            rhs=xch[par][:, nt * chunk_t:(nt + 1) * chunk_t],
                            start=(nt == 0), stop=(nt == JT - 1))
            last.then_inc(pe_tc, 1)
            ve.wait_ge(pe_tc, tc + 1)
            if tc >= 2:
                ve.wait_ge(st_out[par], 16 * (tc // 2))
            lastv = None
            for k in range(K):
                for ic in range(IT):
                    lastv = ve.tensor_scalar_add(
                        pastage[par][:, :, 2 * k + ic],
                        psPA[k * IT + ic][:, :],
                        float(bias_vals[k]))
            lastv.then_inc(dv_tc, 1)
            sp.wait_ge(dv_tc, tc + 1)
            sp.dma_start(out=cdram[tc], in_=pastage[par][:, :, :]
                         ).then_inc(st_out[par], 16)

    # ================= SCAN =================
    psS = [nc.alloc_psum_tensor(f"psS{p}", [128, JT], F32) for p in range(2)]
    psV = [nc.alloc_psum_tensor(f"psV{p}", [128, IT], F32) for p in range(2)]

    po.bir_kernel_barrier_wait([list(range(NCORES))])
    se.sem_inc(s_h, 1)            # prime: loaded h0 counts as "relu(-1)"

    rpe1 = te.alloc_register("rpe1"); te.reg_mov(rpe1, 1)
    rve1 = ve.alloc_register("rve1"); ve.reg_mov(rve1, 1)
    rvl = []
    for p in range(2):
        r = ve.alloc_register(f"rvl{p}"); ve.reg_mov(r, 0); rvl.append(r)
    rve16 = []
    for p in range(2):
        r = ve.alloc_register(f"rve16_{p}"); ve.reg_mov(r, 2 * (NCORES - 1))
        rve16.append(r)
    rq8 = po.alloc_register("rq8"); po.reg_mov(rq8, NCORES - 1)
    rq1 = po.alloc_register("rq1"); po.reg_mov(rq1, 1)
    rql = []
    for p in range(2):
        r = po.alloc_register(f"rql{p}"); po.reg_mov(r, 0); rql.append(r)
    ra1 = se.alloc_register("ra1"); se.reg_mov(ra1, 1)

    ve.wait_ge(ld_sem, 80)
    se.wait_ge(ld_sem, 80)
    te.wait_ge(dv_tc, nchunk)     # phase-A psum banks fully consumed

    sp.wait_ge(st_out[0], 16)
    sp.dma_start(out=cbuf[0][:, :], in_=cdram[0]).then_inc(csem[0], 16)

    relu = mybir.ActivationFunctionType.Relu

    def kstep(par, cbuf_cur, cds, k):
        # --- PE: 8 s-matmuls  psS[par][:, jc] += P_k^T h ---
        te.wait_ge(s_h, rpe1)
        lastm = None
        for jc in range(JT):
            for ic in range(IT):
                wtile = ((k * IT + ic) * JT + jc) * 128
                lastm = te.matmul(psS[par][:, jc:jc + 1],
                                  lhsT=ws_s[:, wtile:wtile + 128],
                                  rhs=h[:, ic:ic + 1],
                                  start=(ic == 0), stop=(ic == IT - 1))
        lastm.then_inc(s_sd, 1)
        # --- DVE: hc = h + c_tk ---
        ve.wait_ge(s_h, rve1)
        ve.tensor_add(hcb[par][:, :], h[:, :],
                      cbuf_cur[:, bass.ds(cds + 2 * k, IT)]).then_inc(s_hc, 1)
        # --- PE: identity matmul preloads h+c into psV ---
        te.wait_ge(s_hc, rpe1)
        te.matmul(psV[par][:, :], lhsT=idn[:, :], rhs=hcb[par][:, :],
                  start=True, stop=False)
        # --- DVE: copy s partials to send buffer + own recv slot ---
        ve.wait_ge(s_sd, rve1)
        ve.wait_ge(lsem[par], rvl[par])
        ve.tensor_copy(send[par][:, :], psS[par][:, :]).then_inc(s_sc, 1)
        ve.tensor_copy(recv[par][:, 0:JT], psS[par][:, :]).then_inc(s_sf, 1)
        ve.reg_add(rvl[par], rvl[par], 16 * (NCORES - 1))
        # --- Q7: 8 single-dest broadcasts + trigger ---
        import os as _os
        _ablate = _os.environ.get("LISTA_ABLATE_REMOTE") == "1"
        for j in range(1, NCORES):
            po.remote_dma_broadcast(
                recv[par][:, JT * j:JT * (j + 1)], send[par][:, :],
                remote_sem=rsem[par], local_sem=lsem[par],
                rdests=[((0, 0) if _ablate else (0, j)) if s == j else None
                        for s in range(NCORES)],
            ).then_inc(psem, 1)
        po.wait_ge(psem, rq8)
        po.wait_ge(s_sc, rq1)
        po.trigger_dma(count=NCORES - 1)
        po.reg_add(rq8, rq8, NCORES - 1)
        po.reg_add(rq1, rq1, 1)
        po.reg_add(rql[par], rql[par], 16 * (NCORES - 1))
        # --- DVE: strided one-op reduce of the 8 partials -> u (bf16) ---
        ve.wait_ge(rsem[par], rve16[par])
        ve.wait_ge(s_sf, rve1)
        with nc.allow_low_precision("u is consumed in bf16 by the PE anyway"):
            ve.tensor_reduce(ubuf[par][:, :],
                             recv[par][:, :].rearrange("p (s j) -> p j s", s=8),
                             mybir.AxisListType.X, mybir.AluOpType.add
                             ).then_inc(s_u, 1)
        ve.reg_add(rve1, rve1, 1)
        ve.reg_add(rve16[par], rve16[par], 2 * (NCORES - 1))
        # --- PE: 8 v-matmuls  psV[:, icol] -= (A_k/a)[icol] u ---
        te.wait_ge(s_u, rpe1)
        lastv = None
        for jc in range(JT):
            for icol in range(IT):
                wtile = ((k * JT + jc) * IT + icol) * 128
                lastv = te.matmul(psV[par][:, icol:icol + 1],
                                  lhsT=ws_v[:, wtile:wtile + 128],
                                  rhs=ubuf[par][:, jc:jc + 1],
                                  start=False,
                                  stop=(jc == JT - 1 and icol == IT - 1))
        lastv.then_inc(s_vd, 1)
        te.reg_add(rpe1, rpe1, 1)
        # --- ACT: h = relu(psV) ---
        se.wait_ge(s_vd, ra1)
        se.activation(h[:, :], psV[par][:, :], relu).then_inc(s_h, 1)
        se.reg_add(ra1, ra1, 1)

    for c in range(nchunk):
        cpar = c % 2
        if c + 1 < nchunk:
            sp.wait_ge(st_out[(c + 1) % 2], 16 * ((c + 1) // 2 + 1))
            if c >= 1:
                sp.wait_ge(s_hc, 3 * chunk_t * c)
            sp.dma_start(out=cbuf[(c + 1) % 2][:, :],
                         in_=cdram[c + 1]).then_inc(csem[(c + 1) % 2], 16)
        ve.wait_ge(csem[cpar], 16 * (c // 2 + 1))
        if c >= 2:
            se.wait_ge(osem[cpar], 16 * (c // 2))
        ost = ostage[cpar]
        with nc.Fori(0, n_body, engines=[PE, DVE, ACT, POOL]) as i:
            for tt in range(2):
                cds = i * (4 * K) + tt * (2 * K)
                for k in range(K):
                    kstep((tt * K + k) % 2, cbuf[cpar], cds, k)
                se.activation(ost[:, bass.ds(i * (2 * IT) + tt * IT, IT)],
                              psV[(tt * K + K - 1) % 2][:, :], relu
                              ).then_inc(s_oc, 1)
        sp.wait_ge(s_oc, chunk_t * (c + 1))
        sp.dma_start(out=hs[c], in_=ost[:, :]).then_inc(osem[cpar], 16)

    for p in range(2):
        sp.wait_ge(osem[p], 16 * ((nchunk + 1 - p) // 2))
    for p in range(2):
        po.wait_ge(lsem[p], rql[p])   # drain outbound broadcasts before exit

    nc.compile()
    return nc


def host_prep(x, A, alpha, h0, t_total, chunk_t):
    import ml_dtypes
    bf = ml_dtypes.bfloat16
    a = np.asarray(alpha[1:, 0, 0], np.float64)

    xTn = np.ascontiguousarray(
        x[:t_total].T.reshape(JT, 128, t_total)).astype(np.float32)
    identity = np.eye(128).astype(bf)

    in_maps = []
    for c in range(NCORES):
        Asl = A[:, c * MSL:(c + 1) * MSL, :]
        wAc = np.zeros((128, K * JT * IT * 128), np.float32)
        wSc = np.zeros((128, K * IT * JT * 128), bf)
        wVc = np.zeros((128, K * JT * IT * 128), bf)
        for k in range(K):
            for nt in range(JT):
                for ic in range(IT):
                    t0 = ((k * JT + nt) * IT + ic) * 128
                    blk = Asl[k, ic * 128:(ic + 1) * 128,
                              nt * 128:(nt + 1) * 128] / a[k]
                    wAc[:, t0:t0 + 128] = blk.T.astype(np.float32)
            for ic in range(IT):
                for jc in range(JT):
                    t0 = ((k * IT + ic) * JT + jc) * 128
                    wSc[:, t0:t0 + 128] = Asl[k, ic * 128:(ic + 1) * 128,
                                              jc * 128:(jc + 1) * 128].astype(bf)
            for jc in range(JT):
                for icol in range(IT):
                    t0 = ((k * JT + jc) * IT + icol) * 128
                    blk = -(Asl[k, icol * 128:(icol + 1) * 128,
                                jc * 128:(jc + 1) * 128] / a[k])
                    wVc[:, t0:t0 + 128] = blk.T.astype(bf)
        h0sl = h0[c * MSL:(c + 1) * MSL, 0].reshape(IT, 128).T.astype(bf)
        in_maps.append({
            "xT": xTn, "wA": wAc, "wS": np.asarray(wSc), "wV": np.asarray(wVc),
            "ident": identity, "h0c": np.ascontiguousarray(h0sl),
        })
    bias_vals = [-RHO / a[k] for k in range(K)]
    return in_maps, bias_vals


def gather_output(results, t_total, chunk_t):
    nchunk = t_total // chunk_t
    out = np.zeros((t_total, M), np.float32)
    for c in range(NCORES):
        hsd = np.asarray(results[c]["hs"]).astype(np.float32)
        hsd = hsd.reshape(nchunk, 128, chunk_t, IT)
        blk = hsd.transpose(0, 2, 3, 1).reshape(t_total, MSL)
        out[:, c * MSL:(c + 1) * MSL] = blk
    return out


def kernel(x, A, alpha, h0, _t_total=T, _chunk_t=512, _trace=False):
    x = np.asarray(x); A = np.asarray(A)
    alpha = np.asarray(alpha); h0 = np.asarray(h0)
    in_maps, bias_vals = host_prep(x, A, alpha, h0, _t_total, _chunk_t)
    nc = build_program(_t_total, _chunk_t, bias_vals)
    from concourse.bass_utils import run_bass_kernel_spmd
    res = run_bass_kernel_spmd(nc, in_maps, list(range(NCORES)), trace=False)
    out = gather_output(res.results, _t_total, _chunk_t)
    kernel.last_exec_ns = res.exec_time_ns
    kernel.last_results = res
    if _trace:
        # no NTFF hook in this container: estimate exec by timing warm reruns
        import time
        from concourse import bass2jax
        times = []
        for _ in range(3):
            t0 = time.perf_counter()
            bass2jax.run_bass_via_pjrt(nc, in_maps, n_cores=NCORES)
            times.append(time.perf_counter() - t0)
        kernel.last_exec_ns = int(min(times) * 1e9)
        kernel.warm_times = times
    return out.astype(np.float32)



# revision 2
# speedup vs baseline: 6.2926x; 6.2926x over previous
"""LISTA scan kernel for 8 TRN2 NeuronCores.

Strategy (tensor-parallel over the m=2048 code dimension, 256 rows/core):
  - B_k h is computed factored as A_k (A_k^T h)  (rank-512, half the FLOPs).
  - Per k-step:  s_c = P_k^T h_c  (PE, bf16)  ->  all-gather of the [512]
    partial via 8 single-dest remote_dma_broadcasts (XOR-relative routing,
    all compile-time-constant addresses)  ->  DVE 3-op tree-sum -> u (bf16)
    ->  psum_v = I @ (h + c_tk) - (A_k/a) u  (PE)  ->  h' = relu(psum_v) (ACT).
  - Phase A precomputes c[t,k,:] = (x @ (A_k/a_k)^T) - rho/a_k on device
    as a plain GEMM, stored to DRAM in the scan's per-partition layout.
  - Raw bacc, manual semaphores, hardware Fori loop per 512-t chunk with
    2 timesteps unrolled per iteration.
"""
import sys
import numpy as np

sys.path.insert(0, '/opt/trn_rl_repo')

from concourse import bass, bacc, mybir  # noqa: E402

T, N, M, K = 8192, 512, 2048, 3
RHO = 1e-4
NCORES = 8
MSL = M // NCORES            # 256 rows per core
JT = N // 128                # 4 u tiles
IT = MSL // 128              # 2 h tiles per core

F32 = mybir.dt.float32
BF16 = mybir.dt.bfloat16


def build_program(t_total, chunk_t, bias_vals, repeats=1):
    nchunk = t_total // chunk_t
    total_chunks = nchunk * repeats
    n_body = chunk_t // 2

    PE = mybir.EngineType.PE
    DVE = mybir.EngineType.DVE
    ACT = mybir.EngineType.Activation
    POOL = mybir.EngineType.Pool

    nc = bacc.Bacc(None, target_bir_lowering=False)

    # ---------------- DRAM ----------------
    xT = nc.declare_dram_parameter("xT", [JT, 128, t_total], F32, isOutput=False)
    wA = nc.declare_dram_parameter("wA", [128, K * JT * IT * 128], F32, isOutput=False)
    wS = nc.declare_dram_parameter("wS", [128, K * IT * JT * 128], BF16, isOutput=False)
    wV = nc.declare_dram_parameter("wV", [128, K * JT * IT * 128], BF16, isOutput=False)
    ident = nc.declare_dram_parameter("ident", [128, 128], BF16, isOutput=False)
    h0c = nc.declare_dram_parameter("h0c", [128, IT], BF16, isOutput=False)
    hs = nc.declare_dram_parameter("hs", [nchunk, 128, chunk_t * IT], BF16,
                                   isOutput=True)
    cdram = nc.dram_tensor("cdram", [nchunk, 128, chunk_t * 2 * K], BF16)

    # ---------------- SBUF ----------------
    ws_s = nc.alloc_sbuf_tensor("ws_s", [128, K * IT * JT * 128], BF16)
    ws_v = nc.alloc_sbuf_tensor("ws_v", [128, K * JT * IT * 128], BF16)
    wa_s = nc.alloc_sbuf_tensor("wa_s", [128, K * JT * IT * 128], F32)
    idn = nc.alloc_sbuf_tensor("idn", [128, 128], BF16)
    h = nc.alloc_sbuf_tensor("h", [128, IT], BF16)
    xch = [nc.alloc_sbuf_tensor(f"xch{p}", [128, JT * chunk_t], F32)
           for p in range(2)]
    pastage = [nc.alloc_sbuf_tensor(f"pastage{p}", [128, chunk_t, 2 * K], BF16)
               for p in range(2)]
    cbuf = [nc.alloc_sbuf_tensor(f"cbuf{p}", [128, chunk_t * 2 * K], BF16)
            for p in range(2)]
    hcb = [nc.alloc_sbuf_tensor(f"hc{p}", [128, IT], BF16) for p in range(2)]
    send = [nc.alloc_sbuf_tensor(f"send{p}", [128, JT], F32) for p in range(2)]
    recv = [nc.alloc_sbuf_tensor(f"recv{p}", [128, NCORES * JT], F32)
            for p in range(2)]
    ubuf = [nc.alloc_sbuf_tensor(f"ubuf{p}", [128, JT], BF16) for p in range(2)]
    ostage = [nc.alloc_sbuf_tensor(f"ostage{p}", [128, chunk_t * IT], BF16)
              for p in range(2)]

    # ---------------- semaphores ----------------
    names = ["ld_sem", "pe_tc", "dv_tc", "psem", "s_h", "s_hc",
             "s_u", "s_sd", "s_vd", "s_sc", "s_oc", "s_sf"]
    sems = {n: nc.alloc_semaphore(n) for n in names}
    (ld_sem, pe_tc, dv_tc, psem, s_h, s_hc, s_u, s_sd, s_vd,
     s_sc, s_oc, s_sf) = (sems[n] for n in names)
    lsem = [nc.alloc_semaphore(f"lsem{p}") for p in range(2)]
    # parity-split sems: multiple unordered DMA producers would otherwise
    # make thresholds ambiguous (a fast producer of round K+2 could satisfy
    # a round-K wait).
    xdma = [nc.alloc_semaphore(f"xdma{p}") for p in range(2)]
    st_out = [nc.alloc_semaphore(f"st_out{p}") for p in range(2)]
    csem = [nc.alloc_semaphore(f"csem{p}") for p in range(2)]
    osem = [nc.alloc_semaphore(f"osem{p}") for p in range(2)]
    rsem = [nc.alloc_semaphore(f"rsem{p}") for p in range(2)]

    te, ve, se, po, sp = nc.tensor, nc.vector, nc.scalar, nc.gpsimd, nc.sync

    # ---------------- entry loads ----------------
    sp.dma_start(out=ws_s[:, :], in_=wS[:, :]).then_inc(ld_sem, 16)
    sp.dma_start(out=ws_v[:, :], in_=wV[:, :]).then_inc(ld_sem, 16)
    sp.dma_start(out=wa_s[:, :], in_=wA[:, :]).then_inc(ld_sem, 16)
    sp.dma_start(out=idn[:, :], in_=ident[:, :]).then_inc(ld_sem, 16)
    sp.dma_start(out=h[:, :], in_=h0c[:, :]).then_inc(ld_sem, 16)

    # ================= PHASE A =================
    import contextlib
    with contextlib.ExitStack() as stack:
        psPA = [stack.enter_context(
            nc.psum_tensor(f"psPA{q}", [128, chunk_t], F32)) for q in range(6)]
        te.wait_ge(ld_sem, 80)
        for tc in range(nchunk):
            par = tc % 2
            if tc >= 2:
                sp.wait_ge(pe_tc, tc - 1)
            for nt in range(JT):
                sp.dma_start(out=xch[par][:, nt * chunk_t:(nt + 1) * chunk_t],
                             in_=xT[nt, :, tc * chunk_t:(tc + 1) * chunk_t]
                             ).then_inc(xdma[par], 16)
            te.wait_ge(xdma[par], 64 * (tc // 2 + 1))
            if tc >= 1:
                te.wait_ge(dv_tc, tc)
            last = None
            for k in range(K):
                for ic in range(IT):
                    for nt in range(JT):
                        wtile = ((k * JT + nt) * IT + ic) * 128
                        last = te.matmul(
                            psPA[k * IT + ic][:, :],
                            lhsT=wa_s[:, wtile:wtile + 128],
                            rhs=xch[par][:, nt * chunk_t:(nt + 1) * chunk_t],
                            start=(nt == 0), stop=(nt == JT - 1))
            last.then_inc(pe_tc, 1)
            ve.wait_ge(pe_tc, tc + 1)
            if tc >= 2:
                ve.wait_ge(st_out[par], 16 * (tc // 2))
            lastv = None
            for k in range(K):
                for ic in range(IT):
                    lastv = ve.tensor_scalar_add(
                        pastage[par][:, :, 2 * k + ic],
                        psPA[k * IT + ic][:, :],
                        float(bias_vals[k]))
            lastv.then_inc(dv_tc, 1)
            sp.wait_ge(dv_tc, tc + 1)
            sp.dma_start(out=cdram[tc], in_=pastage[par][:, :, :]
                         ).then_inc(st_out[par], 16)

    # ================= SCAN =================
    psS = [nc.alloc_psum_tensor(f"psS{p}", [128, JT], F32) for p in range(2)]
    psV = [nc.alloc_psum_tensor(f"psV{p}", [128, IT], F32) for p in range(2)]

    po.bir_kernel_barrier_wait([list(range(NCORES))])
    se.sem_inc(s_h, 1)            # prime: loaded h0 counts as "relu(-1)"

    rpe1 = te.alloc_register("rpe1"); te.reg_mov(rpe1, 1)
    rve1 = ve.alloc_register("rve1"); ve.reg_mov(rve1, 1)
    rvl = []
    for p in range(2):
        r = ve.alloc_register(f"rvl{p}"); ve.reg_mov(r, 0); rvl.append(r)
    rve16 = []
    for p in range(2):
        r = ve.alloc_register(f"rve16_{p}"); ve.reg_mov(r, 2 * (NCORES - 1))
        rve16.append(r)
    rq8 = po.alloc_register("rq8"); po.reg_mov(rq8, NCORES - 1)
    rq1 = po.alloc_register("rq1"); po.reg_mov(rq1, 1)
    rql = []
    for p in range(2):
        r = po.alloc_register(f"rql{p}"); po.reg_mov(r, 0); rql.append(r)
    ra1 = se.alloc_register("ra1"); se.reg_mov(ra1, 1)

    ve.wait_ge(ld_sem, 80)
    se.wait_ge(ld_sem, 80)
    te.wait_ge(dv_tc, nchunk)     # phase-A psum banks fully consumed

    sp.wait_ge(st_out[0], 16)
    sp.dma_start(out=cbuf[0][:, :], in_=cdram[0]).then_inc(csem[0], 16)

    relu = mybir.ActivationFunctionType.Relu

    def kstep(par, cbuf_cur, cds, k):
        # --- PE: 8 s-matmuls  psS[par][:, jc] += P_k^T h ---
        te.wait_ge(s_h, rpe1)
        lastm = None
        for jc in range(JT):
            for ic in range(IT):
                wtile = ((k * IT + ic) * JT + jc) * 128
                lastm = te.matmul(psS[par][:, jc:jc + 1],
                                  lhsT=ws_s[:, wtile:wtile + 128],
                                  rhs=h[:, ic:ic + 1],
                                  start=(ic == 0), stop=(ic == IT - 1))
        lastm.then_inc(s_sd, 1)
        # --- DVE: hc = h + c_tk ---
        ve.wait_ge(s_h, rve1)
        ve.tensor_add(hcb[par][:, :], h[:, :],
                      cbuf_cur[:, bass.ds(cds + 2 * k, IT)]).then_inc(s_hc, 1)
        # --- PE: identity matmul preloads h+c into psV ---
        te.wait_ge(s_hc, rpe1)
        te.matmul(psV[par][:, :], lhsT=idn[:, :], rhs=hcb[par][:, :],
                  start=True, stop=False)
        # --- DVE: copy s partials to send buffer + own recv slot ---
        ve.wait_ge(s_sd, rve1)
        ve.wait_ge(lsem[par], rvl[par])
        ve.tensor_copy(send[par][:, :], psS[par][:, :]).then_inc(s_sc, 1)
        ve.tensor_copy(recv[par][:, 0:JT], psS[par][:, :]).then_inc(s_sf, 1)
        ve.reg_add(rvl[par], rvl[par], 16 * (NCORES - 1))
        # --- Q7: 8 single-dest broadcasts + trigger ---
        import os as _os
        _ablate = _os.environ.get("LISTA_ABLATE_REMOTE") == "1"
        for j in range(1, NCORES):
            po.remote_dma_broadcast(
                recv[par][:, JT * j:JT * (j + 1)], send[par][:, :],
                remote_sem=rsem[par], local_sem=lsem[par],
                rdests=[((0, 0) if _ablate else (0, j)) if s == j else None
                        for s in range(NCORES)],
            ).then_inc(psem, 1)
        po.wait_ge(psem, rq8)
        po.wait_ge(s_sc, rq1)
        po.trigger_dma(count=NCORES - 1)
        po.reg_add(rq8, rq8, NCORES - 1)
        po.reg_add(rq1, rq1, 1)
        po.reg_add(rql[par], rql[par], 16 * (NCORES - 1))
        # --- DVE: strided one-op reduce of the 8 partials -> u (bf16) ---
        ve.wait_ge(rsem[par], rve16[par])
        ve.wait_ge(s_sf, rve1)
        with nc.allow_low_precision("u is consumed in bf16 by the PE anyway"):
            ve.tensor_reduce(ubuf[par][:, :],
                             recv[par][:, :].rearrange("p (s j) -> p j s", s=8),
                             mybir.AxisListType.X, mybir.AluOpType.add
                             ).then_inc(s_u, 1)
        ve.reg_add(rve1, rve1, 1)
        ve.reg_add(rve16[par], rve16[par], 2 * (NCORES - 1))
        # --- PE: 8 v-matmuls  psV[:, icol] -= (A_k/a)[icol] u ---
        te.wait_ge(s_u, rpe1)
        lastv = None
        for jc in range(JT):
            for icol in range(IT):
                wtile = ((k * JT + jc) * IT + icol) * 128
                lastv = te.matmul(psV[par][:, icol:icol + 1],
                                  lhsT=ws_v[:, wtile:wtile + 128],
                                  rhs=ubuf[par][:, jc:jc + 1],
                                  start=False,
                                  stop=(jc == JT - 1 and icol == IT - 1))
        lastv.then_inc(s_vd, 1)
        te.reg_add(rpe1, rpe1, 1)
        # --- ACT: h = relu(psV) ---
        se.wait_ge(s_vd, ra1)
        se.activation(h[:, :], psV[par][:, :], relu).then_inc(s_h, 1)
        se.reg_add(ra1, ra1, 1)

    for c in range(nchunk):
        cpar = c % 2
        if c + 1 < nchunk:
            sp.wait_ge(st_out[(c + 1) % 2], 16 * ((c + 1) // 2 + 1))
            if c >= 1:
                sp.wait_ge(s_hc, 3 * chunk_t * c)
            sp.dma_start(out=cbuf[(c + 1) % 2][:, :],
                         in_=cdram[c + 1]).then_inc(csem[(c + 1) % 2], 16)
        ve.wait_ge(csem[cpar], 16 * (c // 2 + 1))
        if c >= 2:
            se.wait_ge(osem[cpar], 16 * (c // 2))
        ost = ostage[cpar]
        with nc.Fori(0, n_body, engines=[PE, DVE, ACT, POOL]) as i:
            for tt in range(2):
                cds = i * (4 * K) + tt * (2 * K)
                for k in range(K):
                    kstep((tt * K + k) % 2, cbuf[cpar], cds, k)
                se.activation(ost[:, bass.ds(i * (2 * IT) + tt * IT, IT)],
                              psV[(tt * K + K - 1) % 2][:, :], relu
                              ).then_inc(s_oc, 1)
        sp.wait_ge(s_oc, chunk_t * (c + 1))
        sp.dma_start(out=hs[c], in_=ost[:, :]).then_inc(osem[cpar], 16)

    for p in range(2):
        sp.wait_ge(osem[p], 16 * ((nchunk + 1 - p) // 2))
    for p in range(2):
        po.wait_ge(lsem[p], rql[p])   # drain outbound broadcasts before exit

    nc.compile()
    return nc


def host_prep(x, A, alpha, h0, t_total, chunk_t):
    import ml_dtypes
    bf = ml_dtypes.bfloat16
    a = np.asarray(alpha[1:, 0, 0], np.float64)

    xTn = np.ascontiguousarray(
        x[:t_total].T.reshape(JT, 128, t_total)).astype(np.float32)
    identity = np.eye(128).astype(bf)

    in_maps = []
    for c in range(NCORES):
        Asl = A[:, c * MSL:(c + 1) * MSL, :]
        wAc = np.zeros((128, K * JT * IT * 128), np.float32)
        wSc = np.zeros((128, K * IT * JT * 128), bf)
        wVc = np.zeros((128, K * JT * IT * 128), bf)
        for k in range(K):
            for nt in range(JT):
                for ic in range(IT):
                    t0 = ((k * JT + nt) * IT + ic) * 128
                    blk = Asl[k, ic * 128:(ic + 1) * 128,
                              nt * 128:(nt + 1) * 128] / a[k]
                    wAc[:, t0:t0 + 128] = blk.T.astype(np.float32)
            for ic in range(IT):
                for jc in range(JT):
                    t0 = ((k * IT + ic) * JT + jc) * 128
                    wSc[:, t0:t0 + 128] = Asl[k, ic * 128:(ic + 1) * 128,
                                              jc * 128:(jc + 1) * 128].astype(bf)
            for jc in range(JT):
                for icol in range(IT):
                    t0 = ((k * JT + jc) * IT + icol) * 128
                    blk = -(Asl[k, icol * 128:(icol + 1) * 128,
                                jc * 128:(jc + 1) * 128] / a[k])
                    wVc[:, t0:t0 + 128] = blk.T.astype(bf)
        h0sl = h0[c * MSL:(c + 1) * MSL, 0].reshape(IT, 128).T.astype(bf)
        in_maps.append({
            "xT": xTn, "wA": wAc, "wS": np.asarray(wSc), "wV": np.asarray(wVc),
            "ident": identity, "h0c": np.ascontiguousarray(h0sl),
        })
    bias_vals = [-RHO / a[k] for k in range(K)]
    return in_maps, bias_vals


def gather_output(results, t_total, chunk_t):
    nchunk = t_total // chunk_t
    out = np.zeros((t_total, M), np.float32)
    for c in range(NCORES):
        hsd = np.asarray(results[c]["hs"]).astype(np.float32)
        hsd = hsd.reshape(nchunk, 128, chunk_t, IT)
        blk = hsd.transpose(0, 2, 3, 1).reshape(t_total, MSL)
        out[:, c * MSL:(c + 1) * MSL] = blk
    return out


def kernel(x, A, alpha, h0, _t_total=T, _chunk_t=512, _trace=False):
    x = np.asarray(x); A = np.asarray(A)
    alpha = np.asarray(alpha); h0 = np.asarray(h0)
    in_maps, bias_vals = host_prep(x, A, alpha, h0, _t_total, _chunk_t)
    nc = build_program(_t_total, _chunk_t, bias_vals)
    from concourse.bass_utils import run_bass_kernel_spmd
    res = run_bass_kernel_spmd(nc, in_maps, list(range(NCORES)), trace=False)
    out = gather_output(res.results, _t_total, _chunk_t)
    kernel.last_exec_ns = res.exec_time_ns
    kernel.last_results = res
    if _trace:
        # no NTFF hook in this container: estimate exec by timing warm reruns
        import time
        from concourse import bass2jax
        times = []
        for _ in range(3):
            t0 = time.perf_counter()
            bass2jax.run_bass_via_pjrt(nc, in_maps, n_cores=NCORES)
            times.append(time.perf_counter() - t0)
        kernel.last_exec_ns = int(min(times) * 1e9)
        kernel.warm_times = times
    return out.astype(np.float32)

